# revision 5
# baseline (speedup 1.0000x reference)
import sys

sys.path.insert(0, "/opt/trn_rl_repo")

import numpy as np

import concourse.bacc as bacc
import concourse.mybir as mybir
import concourse.tile as tile
from concourse.bass_utils import run_bass_kernel_spmd

F32 = mybir.dt.float32
F32R = mybir.dt.float32r

B, L, C, H, D = 4, 1024, 768, 12, 64
LQ = 512  # query rows per core (batch b = core//2, half = core%2)
NT = C // 128  # 6 tiles over channel dim
KTN = L // 128  # 8 tiles over key dim

USE_F32R = False


def _r(ap):
    return ap.bitcast(F32R) if USE_F32R else ap


_CACHE = {}


def _build():
    nc = bacc.Bacc("TRN2", target_bir_lowering=False, debug=False, num_devices=8)
    din = {}

    def inp(name, shape):
        din[name] = nc.dram_tensor(name, shape, F32, kind="ExternalInput").ap()

    inp("xqT", [C, LQ])
    inp("xkvT", [C, L])
    inp("Wq", [C, C])
    inp("Wk", [C, C])
    inp("Wv", [C, C])
    inp("Wout", [C, C])
    inp("wpre", [C, H])
    inp("wpost", [C, H])
    inp("ones", [128, 128])
    outT = nc.dram_tensor("outT", [C, LQ], F32, kind="ExternalOutput").ap()

    EXP = mybir.ActivationFunctionType.Exp

    with tile.TileContext(nc) as tc:
        with (
            tc.tile_pool(name="persist", bufs=1) as pp,
            tc.tile_pool(name="proj", bufs=1) as proj,
            tc.tile_pool(name="work", bufs=1) as wp,
            tc.tile_pool(name="work2", bufs=2) as wp2,
            tc.tile_pool(name="ps", bufs=2, space="PSUM") as psp,
        ):
            ones_sb = pp.tile([128, 128], F32, tag="ones")
            nc.sync.dma_start(ones_sb[:], din["ones"][:, :])
            wpre_sb = []
            wpost_sb = []
            for t in range(NT):
                wa = pp.tile([128, H], F32, tag=f"wpre{t}")
                wb = pp.tile([128, H], F32, tag=f"wpost{t}")
                nc.sync.dma_start(wa[:], din["wpre"][128 * t : 128 * (t + 1), :])
                nc.sync.dma_start(wb[:], din["wpost"][128 * t : 128 * (t + 1), :])
                wpre_sb.append(wa)
                wpost_sb.append(wb)

            QT = [pp.tile([128, LQ], F32, tag=f"qt{t}", name=f"qt{t}") for t in range(NT)]
            KTs = [pp.tile([128, L], F32, tag=f"kt{t}", name=f"kt{t}") for t in range(NT)]
            V = [pp.tile([128, C], F32, tag=f"v{t}", name=f"v{t}") for t in range(KTN)]
            Wout_sb = [pp.tile([128, C], F32, tag=f"wo{t}", name=f"wo{t}") for t in range(NT)]
            sco = [pp.tile([128, LQ], F32, tag=f"sc{t}", name=f"sc{t}") for t in range(NT)]
            for t in range(NT):
                nc.sync.dma_start(Wout_sb[t][:], din["Wout"][128 * t : 128 * (t + 1), :])

            # inputs (transposed on host): xqT [C, LQ], xkvT [C, L]
            xqT = []
            xkvT = []
            for t in range(NT):
                xa = proj.tile([128, LQ], F32, tag=f"xq{t}")
                xb = proj.tile([128, L], F32, tag=f"xkv{t}")
                nc.sync.dma_start(xa[:], din["xqT"][128 * t : 128 * (t + 1), :])
                nc.sync.dma_start(xb[:], din["xkvT"][128 * t : 128 * (t + 1), :])
                xqT.append(xa)
                xkvT.append(xb)

            def load_w(name):
                w = []
                for t in range(NT):
                    wt = proj.tile([128, C], F32, tag=f"w{t}")
                    nc.sync.dma_start(wt[:], din[name][128 * t : 128 * (t + 1), :])
                    w.append(wt)
                return w

            # ---- projections ----
            # Q^T[cout, l] = sum_cin Wq[cin, cout] * xqT[cin, l]
            Wq_sb = load_w("Wq")
            for co in range(NT):
                ps = psp.tile([128, LQ], F32, tag="lg")
                for ci in range(NT):
                    nc.tensor.matmul(
                        ps[:],
                        _r(Wq_sb[ci][:, 128 * co : 128 * (co + 1)]),
                        _r(xqT[ci][:]),
                        start=(ci == 0),
                        stop=(ci == NT - 1),
                    )
                nc.vector.tensor_copy(QT[co][:], ps[:])

            # K^T[cout, k] likewise, free dim L split in halves of 512
            Wk_sb = load_w("Wk")
            for co in range(NT):
                for kh in range(2):
                    ps = psp.tile([128, 512], F32, tag="lg")
                    for ci in range(NT):
                        nc.tensor.matmul(
                            ps[:],
                            _r(Wk_sb[ci][:, 128 * co : 128 * (co + 1)]),
                            _r(xkvT[ci][:, 512 * kh : 512 * (kh + 1)]),
                            start=(ci == 0),
                            stop=(ci == NT - 1),
                        )
                    nc.vector.tensor_copy(KTs[co][:, 512 * kh : 512 * (kh + 1)], ps[:])

            # V[k, cout] : lhsT = xkvT slice [cin, ktile], rhs = Wv [cin, cout]
            Wv_sb = load_w("Wv")
            for kt in range(KTN):
                for ch in range(2):
                    ps = psp.tile([128, 384], F32, tag="vps")
                    for ci in range(NT):
                        nc.tensor.matmul(
                            ps[:],
                            _r(xkvT[ci][:, 128 * kt : 128 * (kt + 1)]),
                            _r(Wv_sb[ci][:, 384 * ch : 384 * (ch + 1)]),
                            start=(ci == 0),
                            stop=(ci == NT - 1),
                        )
                    nc.vector.tensor_copy(V[kt][:, 384 * ch : 384 * (ch + 1)], ps[:])

            # ---- attention with talking heads, one output head i at a time ----
            for i in range(H):
                # G_i[cin(h,d), l] = W_pre[h,i] * Q^T  (per-partition scale)
                G = []
                for t in range(NT):
                    g = wp.tile([128, LQ], F32, tag=f"g{t}")
                    nc.vector.tensor_scalar_mul(g[:], QT[t][:], wpre_sb[t][:, i : i + 1])
                    G.append(g)

                A = [wp.tile([128, LQ], F32, tag=f"a{kt}", name=f"a{kt}") for kt in range(KTN)]
                dn = psp.tile([128, LQ], F32, tag="dn")
                for kt in range(KTN):
                    lg = psp.tile([128, LQ], F32, tag="lg")
                    for t in range(NT):
                        nc.tensor.matmul(
                            lg[:],
                            _r(KTs[t][:, 128 * kt : 128 * (kt + 1)]),
                            _r(G[t][:]),
                            start=(t == 0),
                            stop=(t == NT - 1),
                        )
                    # E = exp(logits), PSUM -> SBUF on ScalarE
                    nc.scalar.activation(A[kt][:], lg[:], EXP)
                    # den (replicated over partitions): ones.T @ E, accum over kt
                    nc.tensor.matmul(
                        _r(dn[:]) if False else dn[:],
                        _r(ones_sb[:]),
                        _r(A[kt][:]),
                        start=(kt == 0),
                        stop=(kt == KTN - 1),
                        skip_group_check=True,
                    )
                rec = wp2.tile([128, LQ], F32, tag="rec")
                nc.vector.reciprocal(rec[:], dn[:])
                for kt in range(KTN):
                    nc.vector.tensor_mul(A[kt][:], A[kt][:], rec[:])

                # U_i[(j,d), l] = sum_k V[k,(j,d)] A_i[k,l]; then postmix-accumulate
                for t in range(NT):
                    up = psp.tile([128, LQ], F32, tag="u")
                    for kt in range(KTN):
                        nc.tensor.matmul(
                            up[:],
                            _r(V[kt][:, 128 * t : 128 * (t + 1)]),
                            _r(A[kt][:]),
                            start=(kt == 0),
                            stop=(kt == KTN - 1),
                        )
                    if i == 0:
                        nc.vector.tensor_scalar_mul(
                            sco[t][:], up[:], wpost_sb[t][:, i : i + 1]
                        )
                    else:
                        tmp = wp2.tile([128, LQ], F32, tag="tmp")
                        nc.vector.tensor_scalar_mul(
                            tmp[:], up[:], wpost_sb[t][:, i : i + 1]
                        )
                        nc.vector.tensor_add(sco[t][:], sco[t][:], tmp[:])

            # ---- output projection: outT[cout, l] = sum_(j,d) Wout[(j,d),cout] sco ----
            for co in range(NT):
                ps = psp.tile([128, LQ], F32, tag="lg")
                for t in range(NT):
                    nc.tensor.matmul(
                        ps[:],
                        _r(Wout_sb[t][:, 128 * co : 128 * (co + 1)]),
                        _r(sco[t][:]),
                        start=(t == 0),
                        stop=(t == NT - 1),
                    )
                ot = wp2.tile([128, LQ], F32, tag="ot")
                nc.vector.tensor_copy(ot[:], ps[:])
                nc.sync.dma_start(outT[128 * co : 128 * (co + 1), :], ot[:])

    nc.finalize()
    return nc


def kernel(inputs_q, inputs_kv, Wq, Wk, Wv, Wout, W_pre, W_post):
    inputs_q = np.asarray(inputs_q, np.float32)
    inputs_kv = np.asarray(inputs_kv, np.float32)
    Wq = np.asarray(Wq, np.float32)
    Wk = np.asarray(Wk, np.float32)
    Wv = np.asarray(Wv, np.float32)
    Wout = np.asarray(Wout, np.float32)
    W_pre = np.asarray(W_pre, np.float32)
    W_post = np.asarray(W_post, np.float32)

    if "nc" not in _CACHE:
        _CACHE["nc"] = _build()
    nc = _CACHE["nc"]

    Wq_s = np.ascontiguousarray(Wq / np.sqrt(np.float32(D)))
    wpre = np.ascontiguousarray(np.repeat(W_pre, D, axis=0))  # [(h,d), i]
    wpost = np.ascontiguousarray(np.repeat(W_post, D, axis=1).T)  # [(j,d), i]
    ones = np.ones((128, 128), np.float32)

    in_maps = []
    for c in range(8):
        b, half = c // 2, c % 2
        xq = inputs_q[b, half * LQ : (half + 1) * LQ, :]
        xkv = inputs_kv[b]
        in_maps.append(
            {
                "xqT": np.ascontiguousarray(xq.T),
                "xkvT": np.ascontiguousarray(xkv.T),
                "Wq": Wq_s,
                "Wk": np.ascontiguousarray(Wk),
                "Wv": np.ascontiguousarray(Wv),
                "Wout": np.ascontiguousarray(Wout),
                "wpre": wpre,
                "wpost": wpost,
                "ones": ones,
            }
        )

    res = run_bass_kernel_spmd(nc, in_maps, core_ids=list(range(8)))
    out = np.empty((B, L, C), np.float32)
    for c in range(8):
        b, half = c // 2, c % 2
        out[b, half * LQ : (half + 1) * LQ, :] = np.asarray(res.results[c]["outT"]).T
    return out


if __name__ == "__main__":
    rng = np.random.default_rng(0)
    args = {
        "inputs_q": rng.standard_normal((B, L, C), np.float32),
        "inputs_kv": rng.standard_normal((B, L, C), np.float32),
        "Wq": rng.standard_normal((C, C), np.float32) / 27.7,
        "Wk": rng.standard_normal((C, C), np.float32) / 27.7,
        "Wv": rng.standard_normal((C, C), np.float32) / 27.7,
        "Wout": rng.standard_normal((C, C), np.float32) / 27.7,
        "W_pre": rng.standard_normal((H, H), np.float32) / 3.46,
        "W_post": rng.standard_normal((H, H), np.float32) / 3.46,
    }
    o = kernel(**args)
    print("ok", o.shape, o.dtype)
